# revision 9
# baseline (speedup 1.0000x reference)
"""Trainium2 Bass kernel for nn_PointerProbGenerator.

Computes softmax(where(mask==0, -1e9, 10*tanh((x@Wq+bq)@(k@Wk+bk)^T)/sqrt(128)))
for B=16, Lq=512, Lk=4096, D=128, batch-sharded 2 batches/core across 8 cores.

Math used on device (shift-invariant softmax, no row-max needed since
10*tanh(.)/sqrt(128) is bounded by +-0.884):
    t    = tanh(q @ k^T)                      (ACT, from PSUM)
    t'   = t + madd   where madd in {0,-126}  (gpsimd DMA int8->f32 cast+accum)
    u    = exp(C * t')  with C=10/sqrt(128)   (ACT, fused row-sum accum_out)
           masked entries: exp(C*t - 111.4) -> 0 in fp32 (underflow), exact 0s
    p    = u / sum(u)                         (DVE reciprocal + tensor_scalar)
"""

import os
import sys

if "/opt/trn_rl_repo" not in sys.path:
    sys.path.insert(0, "/opt/trn_rl_repo")

import numpy as np

import concourse.bass as bass
import concourse.tile as tile
from concourse import bacc, mybir
from concourse.bass_utils import run_bass_kernel_spmd

NCORES = 8
BPC = 2          # batches per core
LQ = 512
LK = 4096
D = 128
C_SCALE = 10.0 / float(np.sqrt(128.0))   # 0.88388347...
MASK_MAG = 126                           # additive mask magnitude (int8-safe)
USE_F32R = os.environ.get("KERNEL_F32R", "1") == "1"

F32 = mybir.dt.float32
AF = mybir.ActivationFunctionType
ALU = mybir.AluOpType
MM_DT = mybir.dt.float32r if USE_F32R else F32

_PROGRAM = None
LAST_RESULTS = None


def _build_program():
    # Bacc (not plain Bass): its compile() legalizes multi-sem waits
    # (move_matmul_waits_to_ldweights / event semaphores) which walrus
    # codegen rejects on raw Tile output.
    nc = bacc.Bacc("TRN2", target_bir_lowering=False)

    q_ext = nc.dram_tensor("query", [BPC, LQ, D], F32, kind="ExternalInput")
    k_ext = nc.dram_tensor("key", [BPC, LK, D], F32, kind="ExternalInput")
    m_ext = nc.dram_tensor("mask8", [BPC, LQ, LK], mybir.dt.int8,
                           kind="ExternalInput")
    wq_ext = nc.dram_tensor("Wq", [D, D], F32, kind="ExternalInput")
    bq_ext = nc.dram_tensor("bq", [D, 1], F32, kind="ExternalInput")
    wk_ext = nc.dram_tensor("Wk", [D, D], F32, kind="ExternalInput")
    bk_ext = nc.dram_tensor("bk", [D, 1], F32, kind="ExternalInput")
    id_ext = nc.dram_tensor("ident", [D, D], F32, kind="ExternalInput")
    out_ext = nc.dram_tensor("out", [BPC, LQ, LK], F32, kind="ExternalOutput")

    # [b, p, t, d]: tile t holds rows 128t..128t+127, partition = row-in-tile
    qR = q_ext[:].rearrange("b (t p) d -> b p t d", p=128)
    kR = k_ext[:].rearrange("b (t p) d -> b p t d", p=128)

    with tile.TileContext(nc) as tc:
        from contextlib import ExitStack

        with ExitStack() as ctx:
            consts = ctx.enter_context(tc.tile_pool(name="consts", bufs=1))
            nat = ctx.enter_context(tc.tile_pool(name="nat", bufs=3))
            raw = ctx.enter_context(tc.tile_pool(name="raw", bufs=3))
            qt_pool = ctx.enter_context(tc.tile_pool(name="qt", bufs=2))
            kt_pool = ctx.enter_context(tc.tile_pool(name="kt", bufs=2))
            t_pool = ctx.enter_context(tc.tile_pool(name="trow", bufs=3))
            u_pool = ctx.enter_context(tc.tile_pool(name="urow", bufs=3))
            st_pool = ctx.enter_context(tc.tile_pool(name="stats", bufs=3))
            aux = ctx.enter_context(
                tc.tile_pool(name="aux", bufs=2, space="PSUM"))
            score = ctx.enter_context(
                tc.tile_pool(name="score", bufs=2, space="PSUM"))

            wq_s = consts.tile([D, D], F32)
            wk_s = consts.tile([D, D], F32)
            bq_s = consts.tile([D, 1], F32)
            bk_s = consts.tile([D, 1], F32)
            id_s = consts.tile([D, D], F32)
            nc.sync.dma_start(out=wq_s, in_=wq_ext[:])
            nc.sync.dma_start(out=wk_s, in_=wk_ext[:])
            nc.sync.dma_start(out=bq_s, in_=bq_ext[:])
            nc.sync.dma_start(out=bk_s, in_=bk_ext[:])
            nc.sync.dma_start(out=id_s, in_=id_ext[:])

            def project(w_s, b_s, src_nat, ntiles, dst):
                # src_nat: [128, ntiles, 128] natural tiles (partition=row)
                # dst: [128, ntiles*128] SBUF, dst[e, l] = (x @ W + b)[l, e]
                for g in range(ntiles // 4):
                    auxp = aux.tile([128, 512], F32)
                    for j in range(4):
                        nc.tensor.transpose(
                            auxp[:, j * 128:(j + 1) * 128],
                            src_nat[:, g * 4 + j, :], id_s)
                    xrawT = raw.tile([128, 512], F32)
                    nc.vector.tensor_copy(xrawT, auxp)
                    auxm = aux.tile([128, 512], F32)
                    nc.tensor.matmul(auxm, w_s, xrawT, start=True, stop=True)
                    nc.vector.tensor_scalar_add(
                        dst[:, g * 512:(g + 1) * 512], auxm, b_s)

            def prep(b):
                qnat = nat.tile([128, LQ // 128, D], F32)
                nc.sync.dma_start(out=qnat, in_=qR[b])
                qT = qt_pool.tile([128, LQ], MM_DT)
                project(wq_s, bq_s, qnat, LQ // 128, qT)
                kT = kt_pool.tile([128, LK], MM_DT)
                for kc in range(LK // 512):
                    knat = nat.tile([128, 4, D], F32)
                    nc.sync.dma_start(
                        out=knat, in_=kR[b, :, kc * 4:(kc + 1) * 4, :])
                    project(wk_s, bk_s, knat, 4,
                            kT[:, kc * 512:(kc + 1) * 512])
                return qT, kT

            def stage_a(qT, kT, b, m):
                # matmuls + tanh for row block m; returns trow with mask added
                trow = t_pool.tile([128, LK], F32)
                lhs = qT[:, m * 128:(m + 1) * 128]
                for c in range(LK // 1024):
                    sc = score.tile([128, 1024], F32)
                    for h in range(2):
                        rhs = kT[:, c * 1024 + h * 512:c * 1024 + (h + 1) * 512]
                        nc.tensor.matmul(sc[:, h * 512:(h + 1) * 512],
                                         lhs, rhs, start=True, stop=True)
                    nc.scalar.activation(
                        trow[:, c * 1024:(c + 1) * 1024], sc, AF.Tanh)
                # int8 {0,-126} mask: cast + accumulate into tanh output.
                # HW limit: SWDGE CCE accum DMA hangs above 2048B/partition,
                # so split into <=2048-wide chunks.
                for h0 in range(0, LK, 2048):
                    h1 = min(h0 + 2048, LK)
                    nc.gpsimd.dma_start(
                        out=trow[:, h0:h1],
                        in_=m_ext[b, m * 128:(m + 1) * 128, h0:h1],
                        accum_op=ALU.add)
                return trow

            def stage_b(trow, b, m):
                urow = u_pool.tile([128, LK], F32)
                s4 = st_pool.tile([128, LK // 1024], F32)
                for c in range(LK // 1024):
                    nc.scalar.activation(
                        urow[:, c * 1024:(c + 1) * 1024],
                        trow[:, c * 1024:(c + 1) * 1024],
                        AF.Exp, scale=C_SCALE,
                        accum_out=s4[:, c:c + 1])
                ssum = st_pool.tile([128, 1], F32)
                rcp = st_pool.tile([128, 1], F32)
                nc.vector.tensor_reduce(
                    ssum, s4, axis=mybir.AxisListType.X, op=ALU.add)
                nc.vector.reciprocal(rcp, ssum)
                nc.vector.tensor_scalar_mul(urow, urow, rcp)
                nc.sync.dma_start(
                    out=out_ext[b, m * 128:(m + 1) * 128, :], in_=urow)

            # Software-pipelined emission: keep ACT fed while the mask
            # accum-DMA of the previous row is in flight; emit batch-1 prep
            # mid-stream so its PE work hides under queued ACT work.
            rows = [(b, m) for b in range(BPC) for m in range(LQ // 128)]
            qT, kT = prep(0)
            ctxs = {0: (qT, kT)}
            pend = []  # (trow, b, m)
            for i, (b, m) in enumerate(rows):
                if b not in ctxs:
                    ctxs[b] = prep(b)
                qT, kT = ctxs[b]
                pend.append((stage_a(qT, kT, b, m), b, m))
                if i == 2 and BPC > 1:
                    ctxs[1] = prep(1)
                if len(pend) > 1:
                    stage_b(*pend.pop(0))
            while pend:
                stage_b(*pend.pop(0))

    nc.compile()
    return nc


def _get_program():
    global _PROGRAM
    if _PROGRAM is None:
        _PROGRAM = _build_program()
    return _PROGRAM


def kernel(query, key, mask, Wq, bq, Wk, bk):
    global LAST_RESULTS
    nc = _get_program()

    query = np.ascontiguousarray(np.asarray(query, dtype=np.float32))
    key = np.ascontiguousarray(np.asarray(key, dtype=np.float32))
    mask8 = ((np.asarray(mask) - 1) * MASK_MAG).astype(np.int8)
    wq = np.ascontiguousarray(np.asarray(Wq, dtype=np.float32))
    wk = np.ascontiguousarray(np.asarray(Wk, dtype=np.float32))
    bq2 = np.ascontiguousarray(np.asarray(bq, dtype=np.float32).reshape(D, 1))
    bk2 = np.ascontiguousarray(np.asarray(bk, dtype=np.float32).reshape(D, 1))
    ident = np.eye(D, dtype=np.float32)

    in_maps = []
    for c in range(NCORES):
        sl = slice(c * BPC, (c + 1) * BPC)
        in_maps.append({
            "query": query[sl], "key": key[sl], "mask8": mask8[sl],
            "Wq": wq, "bq": bq2, "Wk": wk, "bk": bk2, "ident": ident,
        })

    LAST_RESULTS = run_bass_kernel_spmd(nc, in_maps, core_ids=list(range(NCORES)))
    return np.concatenate(
        [np.asarray(LAST_RESULTS.results[i]["out"]) for i in range(NCORES)],
        axis=0)


# revision 12
# speedup vs baseline: 1.0083x; 1.0083x over previous
"""Trainium2 Bass kernel for nn_PointerProbGenerator.

Computes softmax(where(mask==0, -1e9, 10*tanh((x@Wq+bq)@(k@Wk+bk)^T)/sqrt(128)))
for B=16, Lq=512, Lk=4096, D=128, batch-sharded 2 batches/core across 8 cores.

Math used on device (shift-invariant softmax, no row-max needed since
10*tanh(.)/sqrt(128) is bounded by +-0.884):
    t    = tanh(q @ k^T)                      (ACT, from PSUM)
    t'   = t + madd   where madd in {0,-126}  (gpsimd DMA int8->f32 cast+accum)
    u    = exp(C * t')  with C=10/sqrt(128)   (ACT, fused row-sum accum_out)
           masked entries: exp(C*t - 111.4) -> 0 in fp32 (underflow), exact 0s
    p    = u / sum(u)                         (DVE reciprocal + tensor_scalar)
"""

import os
import sys

if "/opt/trn_rl_repo" not in sys.path:
    sys.path.insert(0, "/opt/trn_rl_repo")

import numpy as np

import concourse.bass as bass
import concourse.tile as tile
from concourse import bacc, mybir
from concourse.bass_utils import run_bass_kernel_spmd

NCORES = 8
BPC = 2          # batches per core
LQ = 512
LK = 4096
D = 128
C_SCALE = 10.0 / float(np.sqrt(128.0))   # 0.88388347...
MASK_MAG = 126                           # additive mask magnitude (int8-safe)
USE_F32R = os.environ.get("KERNEL_F32R", "0") == "1"

F32 = mybir.dt.float32
AF = mybir.ActivationFunctionType
ALU = mybir.AluOpType
MM_DT = mybir.dt.float32r if USE_F32R else F32

_PROGRAM = None
LAST_RESULTS = None


def _build_program():
    # Bacc (not plain Bass): its compile() legalizes multi-sem waits
    # (move_matmul_waits_to_ldweights / event semaphores) which walrus
    # codegen rejects on raw Tile output.
    nc = bacc.Bacc("TRN2", target_bir_lowering=False)

    q_ext = nc.dram_tensor("query", [BPC, LQ, D], F32, kind="ExternalInput")
    k_ext = nc.dram_tensor("key", [BPC, LK, D], F32, kind="ExternalInput")
    m_ext = nc.dram_tensor("mask8", [BPC, LQ, LK], mybir.dt.int8,
                           kind="ExternalInput")
    wq_ext = nc.dram_tensor("Wq", [D, D], F32, kind="ExternalInput")
    bq_ext = nc.dram_tensor("bq", [D, 1], F32, kind="ExternalInput")
    wk_ext = nc.dram_tensor("Wk", [D, D], F32, kind="ExternalInput")
    bk_ext = nc.dram_tensor("bk", [D, 1], F32, kind="ExternalInput")
    id_ext = nc.dram_tensor("ident", [D, D], F32, kind="ExternalInput")
    out_ext = nc.dram_tensor("out", [BPC, LQ, LK], F32, kind="ExternalOutput")

    # [b, p, t, d]: tile t holds rows 128t..128t+127, partition = row-in-tile
    qR = q_ext[:].rearrange("b (t p) d -> b p t d", p=128)
    kR = k_ext[:].rearrange("b (t p) d -> b p t d", p=128)

    with tile.TileContext(nc) as tc:
        from contextlib import ExitStack

        with ExitStack() as ctx:
            consts = ctx.enter_context(tc.tile_pool(name="consts", bufs=1))
            nat = ctx.enter_context(tc.tile_pool(name="nat", bufs=3))
            raw = ctx.enter_context(tc.tile_pool(name="raw", bufs=3))
            qt_pool = ctx.enter_context(tc.tile_pool(name="qt", bufs=2))
            kt_pool = ctx.enter_context(tc.tile_pool(name="kt", bufs=2))
            t_pool = ctx.enter_context(tc.tile_pool(name="trow", bufs=3))
            msk_pool = ctx.enter_context(tc.tile_pool(name="msk", bufs=3))
            u_pool = ctx.enter_context(tc.tile_pool(name="urow", bufs=3))
            st_pool = ctx.enter_context(tc.tile_pool(name="stats", bufs=3))
            aux = ctx.enter_context(
                tc.tile_pool(name="aux", bufs=2, space="PSUM"))
            score = ctx.enter_context(
                tc.tile_pool(name="score", bufs=2, space="PSUM"))

            wq_s = consts.tile([D, D], F32)
            wk_s = consts.tile([D, D], F32)
            bq_s = consts.tile([D, 1], F32)
            bk_s = consts.tile([D, 1], F32)
            id_s = consts.tile([D, D], F32)
            nc.sync.dma_start(out=wq_s, in_=wq_ext[:])
            nc.sync.dma_start(out=wk_s, in_=wk_ext[:])
            nc.sync.dma_start(out=bq_s, in_=bq_ext[:])
            nc.sync.dma_start(out=bk_s, in_=bk_ext[:])
            nc.sync.dma_start(out=id_s, in_=id_ext[:])

            def project(w_s, b_s, src_nat, ntiles, dst):
                # src_nat: [128, ntiles, 128] natural tiles (partition=row)
                # dst: [128, ntiles*128] SBUF, dst[e, l] = (x @ W + b)[l, e]
                for g in range(ntiles // 4):
                    auxp = aux.tile([128, 512], F32)
                    for j in range(4):
                        nc.tensor.transpose(
                            auxp[:, j * 128:(j + 1) * 128],
                            src_nat[:, g * 4 + j, :], id_s)
                    xrawT = raw.tile([128, 512], F32)
                    nc.vector.tensor_copy(xrawT, auxp)
                    auxm = aux.tile([128, 512], F32)
                    nc.tensor.matmul(auxm, w_s, xrawT, start=True, stop=True)
                    nc.vector.tensor_scalar_add(
                        dst[:, g * 512:(g + 1) * 512], auxm, b_s)

            def prep(b):
                qnat = nat.tile([128, LQ // 128, D], F32)
                nc.sync.dma_start(out=qnat, in_=qR[b])
                qT = qt_pool.tile([128, LQ], MM_DT)
                project(wq_s, bq_s, qnat, LQ // 128, qT)
                kT = kt_pool.tile([128, LK], MM_DT)
                for kc in range(LK // 512):
                    knat = nat.tile([128, 4, D], F32)
                    nc.sync.dma_start(
                        out=knat, in_=kR[b, :, kc * 4:(kc + 1) * 4, :])
                    project(wk_s, bk_s, knat, 4,
                            kT[:, kc * 512:(kc + 1) * 512])
                return qT, kT

            def stage_a(qT, kT, b, m):
                # matmuls + tanh for row block m; returns trow with mask added
                trow = t_pool.tile([128, LK], F32)
                msk = msk_pool.tile([128, LK], mybir.dt.int8)
                nc.sync.dma_start(out=msk, in_=m_ext[b, m * 128:(m + 1) * 128, :])
                lhs = qT[:, m * 128:(m + 1) * 128]
                for c in range(LK // 1024):
                    sc = score.tile([128, 1024], F32)
                    for h in range(2):
                        rhs = kT[:, c * 1024 + h * 512:c * 1024 + (h + 1) * 512]
                        nc.tensor.matmul(sc[:, h * 512:(h + 1) * 512],
                                         lhs, rhs, start=True, stop=True)
                    nc.scalar.activation(
                        trow[:, c * 1024:(c + 1) * 1024], sc, AF.Tanh)
                # int8 {0,-126} mask added on Pool (cast-on-read); cheaper
                # than a SWDGE RMW accum DMA (16MB of SBUF write traffic,
                # which also hangs HW above 2048B/partition).
                nc.gpsimd.tensor_tensor(out=trow, in0=trow, in1=msk,
                                        op=ALU.add)
                return trow

            def stage_b(trow, b, m):
                urow = u_pool.tile([128, LK], F32)
                s4 = st_pool.tile([128, LK // 1024], F32)
                for c in range(LK // 1024):
                    nc.scalar.activation(
                        urow[:, c * 1024:(c + 1) * 1024],
                        trow[:, c * 1024:(c + 1) * 1024],
                        AF.Exp, scale=C_SCALE,
                        accum_out=s4[:, c:c + 1])
                ssum = st_pool.tile([128, 1], F32)
                rcp = st_pool.tile([128, 1], F32)
                nc.vector.tensor_reduce(
                    ssum, s4, axis=mybir.AxisListType.X, op=ALU.add)
                nc.vector.reciprocal(rcp, ssum)
                nc.vector.tensor_scalar_mul(urow, urow, rcp)
                nc.sync.dma_start(
                    out=out_ext[b, m * 128:(m + 1) * 128, :], in_=urow)

            # Software-pipelined emission: keep ACT fed while the mask
            # accum-DMA of the previous row is in flight; emit batch-1 prep
            # mid-stream so its PE work hides under queued ACT work.
            rows = [(b, m) for b in range(BPC) for m in range(LQ // 128)]
            qT, kT = prep(0)
            ctxs = {0: (qT, kT)}
            pend = []  # (trow, b, m)
            for i, (b, m) in enumerate(rows):
                if b not in ctxs:
                    ctxs[b] = prep(b)
                qT, kT = ctxs[b]
                pend.append((stage_a(qT, kT, b, m), b, m))
                if i == 2 and BPC > 1:
                    ctxs[1] = prep(1)
                if len(pend) > 1:
                    stage_b(*pend.pop(0))
            while pend:
                stage_b(*pend.pop(0))

    nc.compile()
    return nc


def _get_program():
    global _PROGRAM
    if _PROGRAM is None:
        _PROGRAM = _build_program()
    return _PROGRAM


def kernel(query, key, mask, Wq, bq, Wk, bk):
    global LAST_RESULTS
    nc = _get_program()

    query = np.ascontiguousarray(np.asarray(query, dtype=np.float32))
    key = np.ascontiguousarray(np.asarray(key, dtype=np.float32))
    mask8 = ((np.asarray(mask) - 1) * MASK_MAG).astype(np.int8)
    wq = np.ascontiguousarray(np.asarray(Wq, dtype=np.float32))
    wk = np.ascontiguousarray(np.asarray(Wk, dtype=np.float32))
    bq2 = np.ascontiguousarray(np.asarray(bq, dtype=np.float32).reshape(D, 1))
    bk2 = np.ascontiguousarray(np.asarray(bk, dtype=np.float32).reshape(D, 1))
    ident = np.eye(D, dtype=np.float32)

    in_maps = []
    for c in range(NCORES):
        sl = slice(c * BPC, (c + 1) * BPC)
        in_maps.append({
            "query": query[sl], "key": key[sl], "mask8": mask8[sl],
            "Wq": wq, "bq": bq2, "Wk": wk, "bk": bk2, "ident": ident,
        })

    LAST_RESULTS = run_bass_kernel_spmd(nc, in_maps, core_ids=list(range(NCORES)))
    return np.concatenate(
        [np.asarray(LAST_RESULTS.results[i]["out"]) for i in range(NCORES)],
        axis=0)


# revision 16
# speedup vs baseline: 1.1021x; 1.0930x over previous
"""Trainium2 Bass kernel for nn_PointerProbGenerator.

Computes softmax(where(mask==0, -1e9, 10*tanh((x@Wq+bq)@(k@Wk+bk)^T)/sqrt(128)))
for B=16, Lq=512, Lk=4096, D=128, batch-sharded 2 batches/core across 8 cores.

Math used on device (shift-invariant softmax, no row-max needed since
10*tanh(.)/sqrt(128) is bounded by +-0.884):
    q',k' = hi/lo bf16 split of projections   (PE transpose + fp32 mm + DVE)
    s    = qh@kh^T + qh@kl^T + ql@kh^T        (PE bf16, ~16-bit mantissa)
    t    = tanh(s)                            (ACT, from PSUM)
    t'   = t + madd   where madd in {0,-126}  (Pool tensor_tensor, int8 cast)
    u    = exp(C * t')  with C=10/sqrt(128)   (ACT, fused row-sum accum_out)
           masked entries: exp(C*t - 111.4) -> 0 in fp32 (underflow), exact 0s
    p    = u / sum(u)                         (DVE reciprocal + tensor_scalar)
"""

import os
import sys

if "/opt/trn_rl_repo" not in sys.path:
    sys.path.insert(0, "/opt/trn_rl_repo")

import numpy as np

import concourse.bass as bass
import concourse.tile as tile
from concourse import bacc, mybir
from concourse.bass_utils import run_bass_kernel_spmd

NCORES = 8
BPC = 2          # batches per core
LQ = 512
LK = 4096
D = 128
C_SCALE = 10.0 / float(np.sqrt(128.0))   # 0.88388347...
MASK_MAG = 126                           # additive mask magnitude (int8-safe)

F32 = mybir.dt.float32
BF16 = mybir.dt.bfloat16
AF = mybir.ActivationFunctionType
ALU = mybir.AluOpType

_PROGRAM = None
LAST_RESULTS = None


def _build_program():
    # Bacc (not plain Bass): its compile() legalizes multi-sem waits
    # (move_matmul_waits_to_ldweights / event semaphores) which walrus
    # codegen rejects on raw Tile output.
    nc = bacc.Bacc("TRN2", target_bir_lowering=False)

    q_ext = nc.dram_tensor("query", [BPC, LQ, D], F32, kind="ExternalInput")
    k_ext = nc.dram_tensor("key", [BPC, LK, D], F32, kind="ExternalInput")
    m_ext = nc.dram_tensor("mask8", [BPC, LQ, LK], mybir.dt.int8,
                           kind="ExternalInput")
    wq_ext = nc.dram_tensor("Wq", [D, D], F32, kind="ExternalInput")
    bq_ext = nc.dram_tensor("bq", [D, 1], F32, kind="ExternalInput")
    wk_ext = nc.dram_tensor("Wk", [D, D], F32, kind="ExternalInput")
    bk_ext = nc.dram_tensor("bk", [D, 1], F32, kind="ExternalInput")
    id_ext = nc.dram_tensor("ident", [D, D], F32, kind="ExternalInput")
    out_ext = nc.dram_tensor("out", [BPC, LQ, LK], F32, kind="ExternalOutput")

    # [b, p, t, d]: tile t holds rows 128t..128t+127, partition = row-in-tile
    qR = q_ext[:].rearrange("b (t p) d -> b p t d", p=128)
    kR = k_ext[:].rearrange("b (t p) d -> b p t d", p=128)

    with tile.TileContext(nc) as tc:
        from contextlib import ExitStack

        with ExitStack() as ctx:
            consts = ctx.enter_context(tc.tile_pool(name="consts", bufs=1))
            nat = ctx.enter_context(tc.tile_pool(name="nat", bufs=3))
            raw = ctx.enter_context(tc.tile_pool(name="raw", bufs=3))
            qt_pool = ctx.enter_context(tc.tile_pool(name="qt", bufs=2))
            kt_pool = ctx.enter_context(tc.tile_pool(name="kt", bufs=2))
            t_pool = ctx.enter_context(tc.tile_pool(name="trow", bufs=3))
            msk_pool = ctx.enter_context(tc.tile_pool(name="msk", bufs=3))
            u_pool = ctx.enter_context(tc.tile_pool(name="urow", bufs=3))
            st_pool = ctx.enter_context(tc.tile_pool(name="stats", bufs=3))
            aux = ctx.enter_context(
                tc.tile_pool(name="aux", bufs=2, space="PSUM"))
            score = ctx.enter_context(
                tc.tile_pool(name="score", bufs=2, space="PSUM"))

            wq_s = consts.tile([D, D], F32)
            wk_s = consts.tile([D, D], F32)
            bq_s = consts.tile([D, 1], F32)
            bk_s = consts.tile([D, 1], F32)
            id_s = consts.tile([D, D], F32)
            nc.sync.dma_start(out=wq_s, in_=wq_ext[:])
            nc.sync.dma_start(out=wk_s, in_=wk_ext[:])
            nc.sync.dma_start(out=bq_s, in_=bq_ext[:])
            nc.sync.dma_start(out=bk_s, in_=bk_ext[:])
            nc.sync.dma_start(out=id_s, in_=id_ext[:])

            def project(w_s, b_s, src_nat, ntiles, dst_h, dst_l):
                # src_nat: [128, ntiles, 128] natural tiles (partition=row)
                # dst_h/dst_l: bf16 hi/lo split of (x @ W + b)^T so score
                # matmuls run at 1 cyc/row instead of fp32 LOW_HIGH ~2.6
                for g in range(ntiles // 4):
                    auxp = aux.tile([128, 512], F32)
                    for j in range(4):
                        nc.tensor.transpose(
                            auxp[:, j * 128:(j + 1) * 128],
                            src_nat[:, g * 4 + j, :], id_s)
                    xrawT = raw.tile([128, 512], F32)
                    nc.vector.tensor_copy(xrawT, auxp)
                    auxm = aux.tile([128, 512], F32)
                    nc.tensor.matmul(auxm, w_s, xrawT, start=True, stop=True)
                    sl = slice(g * 512, (g + 1) * 512)
                    nc.vector.tensor_scalar_add(dst_h[:, sl], auxm, b_s)
                    nc.vector.scalar_tensor_tensor(
                        dst_l[:, sl], in0=auxm, scalar=b_s, in1=dst_h[:, sl],
                        op0=ALU.add, op1=ALU.subtract)

            def prep(b):
                qnat = nat.tile([128, LQ // 128, D], F32)
                nc.sync.dma_start(out=qnat, in_=qR[b])
                qTh = qt_pool.tile([128, LQ], BF16)
                qTl = qt_pool.tile([128, LQ], BF16)
                project(wq_s, bq_s, qnat, LQ // 128, qTh, qTl)
                kTh = kt_pool.tile([128, LK], BF16)
                kTl = kt_pool.tile([128, LK], BF16)
                for kc in range(LK // 512):
                    knat = nat.tile([128, 4, D], F32)
                    nc.sync.dma_start(
                        out=knat, in_=kR[b, :, kc * 4:(kc + 1) * 4, :])
                    sl = slice(kc * 512, (kc + 1) * 512)
                    project(wk_s, bk_s, knat, 4, kTh[:, sl], kTl[:, sl])
                return qTh, qTl, kTh, kTl

            def stage_a(qT, kT, b, m):
                # matmuls + tanh for row block m; returns trow with mask added
                qTh, qTl = qT
                kTh, kTl = kT
                trow = t_pool.tile([128, LK], F32)
                msk = msk_pool.tile([128, LK], mybir.dt.int8)
                nc.sync.dma_start(out=msk, in_=m_ext[b, m * 128:(m + 1) * 128, :])
                lhs_h = qTh[:, m * 128:(m + 1) * 128]
                lhs_l = qTl[:, m * 128:(m + 1) * 128]
                for c in range(LK // 1024):
                    sc = score.tile([128, 1024], F32)
                    for h in range(2):
                        sl = slice(c * 1024 + h * 512, c * 1024 + (h + 1) * 512)
                        so = slice(h * 512, (h + 1) * 512)
                        nc.tensor.matmul(sc[:, so], lhs_h, kTh[:, sl],
                                         start=True, stop=False)
                        nc.tensor.matmul(sc[:, so], lhs_h, kTl[:, sl],
                                         start=False, stop=False)
                        nc.tensor.matmul(sc[:, so], lhs_l, kTh[:, sl],
                                         start=False, stop=True)
                    nc.scalar.activation(
                        trow[:, c * 1024:(c + 1) * 1024], sc, AF.Tanh)
                # int8 {0,-126} mask added on Pool (cast-on-read); cheaper
                # than a SWDGE RMW accum DMA (16MB of SBUF write traffic,
                # which also hangs HW above 2048B/partition).
                nc.gpsimd.tensor_tensor(out=trow, in0=trow, in1=msk,
                                        op=ALU.add)
                return trow

            def stage_b(trow, b, m):
                urow = u_pool.tile([128, LK], F32)
                s4 = st_pool.tile([128, LK // 1024], F32)
                for c in range(LK // 1024):
                    nc.scalar.activation(
                        urow[:, c * 1024:(c + 1) * 1024],
                        trow[:, c * 1024:(c + 1) * 1024],
                        AF.Exp, scale=C_SCALE,
                        accum_out=s4[:, c:c + 1])
                ssum = st_pool.tile([128, 1], F32)
                rcp = st_pool.tile([128, 1], F32)
                nc.vector.tensor_reduce(
                    ssum, s4, axis=mybir.AxisListType.X, op=ALU.add)
                nc.vector.reciprocal(rcp, ssum)
                nc.vector.tensor_scalar_mul(urow, urow, rcp)
                nc.sync.dma_start(
                    out=out_ext[b, m * 128:(m + 1) * 128, :], in_=urow)

            # Software-pipelined emission: keep ACT fed while the mask
            # accum-DMA of the previous row is in flight; emit batch-1 prep
            # mid-stream so its PE work hides under queued ACT work.
            rows = [(b, m) for b in range(BPC) for m in range(LQ // 128)]
            ctxs = {0: prep(0)}
            pend = []  # (trow, b, m)
            for i, (b, m) in enumerate(rows):
                if b not in ctxs:
                    ctxs[b] = prep(b)
                ct = ctxs[b]
                pend.append((stage_a(ct[:2], ct[2:], b, m), b, m))
                if i == 2 and BPC > 1:
                    ctxs[1] = prep(1)
                if len(pend) > 1:
                    stage_b(*pend.pop(0))
            while pend:
                stage_b(*pend.pop(0))

    nc.compile()
    return nc


def _get_program():
    global _PROGRAM
    if _PROGRAM is None:
        _PROGRAM = _build_program()
    return _PROGRAM


def kernel(query, key, mask, Wq, bq, Wk, bk):
    global LAST_RESULTS
    nc = _get_program()

    query = np.ascontiguousarray(np.asarray(query, dtype=np.float32))
    key = np.ascontiguousarray(np.asarray(key, dtype=np.float32))
    mask8 = ((np.asarray(mask) - 1) * MASK_MAG).astype(np.int8)
    wq = np.ascontiguousarray(np.asarray(Wq, dtype=np.float32))
    wk = np.ascontiguousarray(np.asarray(Wk, dtype=np.float32))
    bq2 = np.ascontiguousarray(np.asarray(bq, dtype=np.float32).reshape(D, 1))
    bk2 = np.ascontiguousarray(np.asarray(bk, dtype=np.float32).reshape(D, 1))
    ident = np.eye(D, dtype=np.float32)

    in_maps = []
    for c in range(NCORES):
        sl = slice(c * BPC, (c + 1) * BPC)
        in_maps.append({
            "query": query[sl], "key": key[sl], "mask8": mask8[sl],
            "Wq": wq, "bq": bq2, "Wk": wk, "bk": bk2, "ident": ident,
        })

    LAST_RESULTS = run_bass_kernel_spmd(nc, in_maps, core_ids=list(range(NCORES)))
    return np.concatenate(
        [np.asarray(LAST_RESULTS.results[i]["out"]) for i in range(NCORES)],
        axis=0)
